# revision 3
# baseline (speedup 1.0000x reference)
"""Trainium2 Bass kernel for a dense transformer block (B=2, T=2048, C=1024, H=16).

Sharding (8 NeuronCores, one chip; identical instruction stream per core,
per-core differences enter only through input data):
  - LayerNorms / projections / MLP: token-sharded. 4096 tokens -> 512 per core.
    Core c owns 128-token blocks {c, 15-c} of each batch (causal load balance).
  - Attention: head-sharded. Core c computes heads {c, c+8} for both batches
    over the full causal sequence.
  - Collective glue: one AllGather of h1^T (post-LN1 activations) and a
    per-batch AllToAll of o^T (attention output) so the second batch's
    attention hides the first AllToAll.

Structure notes (vs the v1 baseline, ~900us -> ~570us best observed):
  - qT/kT/vv are stored in natural token order (batch-major), so attention
    scores stream 512 query columns per matmul: ST[s, t-chunk] = K_j^T Q_chunk
    with the two heads packed as row-tiled (tile_position) K=64 pairs.
  - exp runs on [128, 2, 512-moff] PSUM tiles (two banks, both heads) in one
    activation instruction, trimmed to the causal column range; ~80 big exps
    instead of 544 [128,128] ones. The scalar engine was the v1 bottleneck.
  - P^T tiles live in SBUF per (batch, chunk); P@V consumes only causal
    column slices, so non-causal garbage is never read.
  - MLP-down accumulates all 32 k-tiles in 8 persistent PSUM banks (one per
    output (tq,co) tile): 8 residual adds instead of 32.
  - W1 is prefetched during attention; W2 streams in during the MLP with a
    16-tile window; both are emitted on the sync queue after the
    latency-critical gather reads.
  - The first attention chunk of each batch (ST+exp) is emitted inside the
    QKV rank loop (after rank 3) so the scalar engine starts exping while
    the tensor engine finishes QKV; MLP-up is split by batch token-half and
    emitted before the batch-1 projection so PE is not queued in-order
    behind the second AllToAll.

Precision: matmul operands bf16 (fp32 accumulation in PSUM); LayerNorm
statistics, softmax normalization and both residual streams stay fp32.
"""

import sys

if "/opt/trn_rl_repo" not in sys.path:
    sys.path.insert(0, "/opt/trn_rl_repo")

import ml_dtypes
import numpy as np

import concourse.bass as bass
import concourse.mybir as mybir
import concourse.tile as tile
from concourse import bacc
from concourse.bass_utils import run_bass_kernel_spmd

FP = mybir.dt.float32
BF = mybir.dt.bfloat16
NPBF = ml_dtypes.bfloat16
AF = mybir.ActivationFunctionType
ALU = mybir.AluOpType

B, T, C, H, HD = 2, 2048, 1024, 16, 64
NCORE = 8
BLK = 128
NB = T // BLK  # 16 blocks of 128 tokens per batch
OWN = B * T // NCORE  # 512 tokens per core
EPS = 1e-5

# Optional knobs for the local test harness (not used by grader)
TRACE = False
LAST_RESULT = None
SIM_MODE = False  # replace collectives with local DMA copies (TimelineSim)


def _own_blocks(c):
    """Blocks (b, j) owned by core c, in shard-row order."""
    return [(b, j) for b in range(B) for j in (c, NB - 1 - c)]


def _rank_of(j):
    return j if j < NCORE else NB - 1 - j


def _bcast(handle, n_free):
    """AP broadcasting a 1-D DRAM tensor across 128 partitions (DMA only)."""
    ap = handle[:]
    return bass.AP(tensor=ap.tensor, offset=ap.offset, ap=[[0, 128], *ap.ap])


def _layernorm(nc, pool_stats, eps_sb, out_ap, in_ap, g_sb, be_sb):
    """LN over free axis (1024) of a [128, 1024] tile; out may alias in_."""
    x3 = in_ap.rearrange("p (n s) -> p n s", s=512)
    stats = pool_stats.tile([128, 2, 6], FP, tag="bnstats")
    for sg in range(2):
        nc.vector.bn_stats(out=stats[:, sg, :], in_=x3[:, sg, :])
    mv = pool_stats.tile([128, 2], FP, tag="bnaggr")
    nc.vector.bn_aggr(out=mv, in_=stats)
    std = pool_stats.tile([128, 1], FP, tag="std")
    nc.scalar.activation(out=std, in_=mv[:, 1:2], func=AF.Sqrt, bias=eps_sb)
    rstd = pool_stats.tile([128, 1], FP, tag="rstd")
    nc.vector.reciprocal(out=rstd, in_=std)
    nc.vector.tensor_scalar(
        out=out_ap,
        in0=in_ap,
        scalar1=mv[:, 0:1],
        scalar2=rstd,
        op0=ALU.subtract,
        op1=ALU.mult,
    )
    if g_sb is not None:
        nc.vector.tensor_mul(out=out_ap, in0=out_ap, in1=g_sb)
    if be_sb is not None:
        nc.vector.tensor_add(out=out_ap, in0=out_ap, in1=be_sb)


class _Sections:
    """Byte-compatible packing of many logical tensors into one flat DRAM
    tensor (cuts per-call PJRT buffer-binding overhead, ~40us per input)."""

    def __init__(self):
        self.offsets = {}
        self.total = 0

    def add(self, name, shape):
        n = int(np.prod(shape))
        self.offsets[name] = (self.total, tuple(shape))
        # pad each section to a 512-element boundary for DMA alignment
        self.total += (n + 511) // 512 * 512


F32S = _Sections()
F32S.add("x_own", (OWN, C))
F32S.add("b1v", (32, 128))
F32S.add("bproj", (C,))
F32S.add("b2", (C,))
F32S.add("g1", (C,))
F32S.add("be1", (C,))
F32S.add("g2", (C,))
F32S.add("be2", (C,))
F32S.add("ident", (BLK, BLK))

BF16S = _Sections()
BF16S.add("wq", (C, 2 * HD))
BF16S.add("wk", (C, 2 * HD))
BF16S.add("wv", (C, 2 * HD))
BF16S.add("wproj", (C, C))
BF16S.add("w1b", (32, C, 128))
BF16S.add("w2", (4 * C, C))
BF16S.add("utri", (BLK, BLK))


def _sect(pack, sections, name):
    """AP over a packed section, reshaped to its logical 2D/3D shape."""
    off, shape = sections.offsets[name]
    n = int(np.prod(shape))
    flat = pack[off : off + n]
    if len(shape) == 1:
        return flat
    if len(shape) == 2:
        return flat.rearrange("(p m) -> p m", p=shape[0])
    return flat.rearrange("(a p m) -> a p m", a=shape[0], p=shape[1])


def _build(reps=1, ln1_affine=True, ln2_affine=True, add_b2=True, add_bproj=True):
    nc = bacc.Bacc(None, num_devices=NCORE)

    # ---- kernel I/O (per-core data differs, shapes identical) ----
    packf32 = nc.dram_tensor("packf32", [F32S.total], FP, kind="ExternalInput")
    packbf16 = nc.dram_tensor("packbf16", [BF16S.total], BF, kind="ExternalInput")
    x_own = _sect(packf32, F32S, "x_own")
    b1v = _sect(packf32, F32S, "b1v")
    bproj = _sect(packf32, F32S, "bproj")
    b2 = _sect(packf32, F32S, "b2")
    g1 = _sect(packf32, F32S, "g1")
    be1 = _sect(packf32, F32S, "be1")
    g2 = _sect(packf32, F32S, "g2")
    be2 = _sect(packf32, F32S, "be2")
    ident = _sect(packf32, F32S, "ident")
    wq = _sect(packbf16, BF16S, "wq")
    wk = _sect(packbf16, BF16S, "wk")
    wv = _sect(packbf16, BF16S, "wv")
    wproj = _sect(packbf16, BF16S, "wproj")
    w1b = _sect(packbf16, BF16S, "w1b")
    w2 = _sect(packbf16, BF16S, "w2")
    utri = _sect(packbf16, BF16S, "utri")
    out = nc.dram_tensor("out", [OWN, C], FP, kind="ExternalOutput")

    rg = [list(range(NCORE))]

    with tile.TileContext(nc) as tc:
        with (
            tc.tile_pool(name="dram", bufs=1, space="DRAM") as dram,
            tc.tile_pool(name="consts", bufs=1) as consts,
            tc.tile_pool(name="stats", bufs=12) as stats,
            tc.tile_pool(name="resid", bufs=4) as resid,
        ):

            # ---- constants in SBUF ----
            eps_sb = consts.tile([128, 1], FP)
            nc.vector.memset(eps_sb, EPS)
            g1b = be1b = g2b = be2b = None
            if ln1_affine:
                g1b = consts.tile([128, C], FP, name="g1b")
                nc.gpsimd.dma_start(out=g1b, in_=_bcast(g1, C))
                be1b = consts.tile([128, C], FP, name="be1b")
                nc.gpsimd.dma_start(out=be1b, in_=_bcast(be1, C))
            if ln2_affine:
                g2b = consts.tile([128, C], FP, name="g2b")
                nc.gpsimd.dma_start(out=g2b, in_=_bcast(g2, C))
                be2b = consts.tile([128, C], FP, name="be2b")
                nc.gpsimd.dma_start(out=be2b, in_=_bcast(be2, C))
            bprojb = b2b = None
            if add_bproj:
                bprojb = consts.tile([128, C], FP, name="bprojb")
                nc.gpsimd.dma_start(out=bprojb, in_=_bcast(bproj, C))
            if add_b2:
                b2b = consts.tile([128, C], FP, name="b2b")
                nc.gpsimd.dma_start(out=b2b, in_=_bcast(b2, C))
            utri_sb = consts.tile([BLK, BLK], BF)
            nc.sync.dma_start(out=utri_sb, in_=utri[:])
            utri2_sb = consts.tile([BLK, 2 * BLK], BF)
            nc.vector.tensor_copy(out=utri2_sb[:, 0:BLK], in_=utri_sb)
            nc.vector.tensor_copy(out=utri2_sb[:, BLK : 2 * BLK], in_=utri_sb)
            ident_sb = consts.tile([BLK, BLK], FP)
            nc.sync.dma_start(out=ident_sb, in_=ident[:])
            b1_sb = consts.tile([128, 32], FP)
            nc.gpsimd.dma_start(out=b1_sb, in_=b1v[:].rearrange("a p -> p a"))

            def _body(rep):
                HC = C // 2
                # Pools whose tiles outlive the attention section are
                # allocated first (LIFO pool release discipline); their DMAs
                # are emitted later, after the latency-critical gather reads.
                w1_ctx = tc.tile_pool(name="w1s", bufs=16)
                w1_pool = w1_ctx.__enter__()
                wp_ctx = tc.tile_pool(name="wp", bufs=8)
                wp_pool = wp_ctx.__enter__()
                og0_ctx = tc.tile_pool(name="og0", bufs=8)
                og0_pool = og0_ctx.__enter__()
                h1T_shard = dram.tile([C, OWN], BF, name=f"h1T_shard{rep}", tag=f"sh{rep}")
                h1T_gath = dram.tile(
                    [NCORE * C, OWN], BF, name=f"h1T_gath{rep}", tag=f"ga{rep}",
                    addr_space="Local" if SIM_MODE else "Shared",
                )
                # Per-batch AllToAll: a2a_in[b] rows r*128.. = my heads' o^T for
                # rank r's tokens of batch b; a2a_out[b] rows r*128.. = rank r's
                # heads for MY tokens of batch b.
                a2a_in = [
                    dram.tile(
                        [NCORE * BLK, 2 * BLK], BF,
                        name=f"a2a_in{rep}_{b}", tag=f"ai{rep}_{b}",
                    )
                    for b in range(B)
                ]
                a2a_out = [
                    dram.tile(
                        [NCORE * BLK, 2 * BLK], BF,
                        name=f"a2a_out{rep}_{b}", tag=f"ao{rep}_{b}",
                    )
                    for b in range(B)
                ]

                # ================= Phase 1: LN1 on own tokens, h1^T shard =======
                xo_sb = []  # own x tiles; overwritten with x2 (post-attn residual)
                for i in range(4):
                    xo = resid.tile([128, C], FP, tag="xo", name=f"xo{i}")
                    xo_sb.append(xo)
                with (
                    tc.tile_pool(name="hwork", bufs=4) as hwork,
                    tc.tile_pool(name="h1Tp", bufs=8) as h1Tp,
                    tc.tile_pool(name="tp1", bufs=2, space="PSUM") as tp1_ps,
                ):
                    h1T_sb = [
                        h1Tp.tile([128, OWN], BF, tag="h1T", name=f"h1T{ct}")
                        for ct in range(8)
                    ]
                    for i in range(4):
                        h1 = hwork.tile([128, C], FP, tag="h1", name=f"h1_{i}")
                        nc.sync.dma_start(
                            out=xo_sb[i], in_=x_own[i * 128 : (i + 1) * 128, :]
                        )
                        _layernorm(nc, stats, eps_sb, h1[:], xo_sb[i][:], g1b, be1b)
                        for ct in range(8):
                            tp = tp1_ps.tile([128, 128], FP, tag="tp", name="tp1")
                            nc.tensor.transpose(
                                tp, h1[:, ct * 128 : (ct + 1) * 128], ident_sb
                            )
                            dst = h1T_sb[ct][:, i * 128 : (i + 1) * 128]
                            if ct % 2 == 0:
                                nc.vector.tensor_copy(out=dst, in_=tp)
                            else:
                                nc.scalar.copy(out=dst, in_=tp)
                    for ct in range(8):
                        nc.sync.dma_start(
                            out=h1T_shard[ct * 128 : (ct + 1) * 128, :], in_=h1T_sb[ct]
                        )

                # ================= Phase 2: AllGather h1^T ======================
                if SIM_MODE:
                    for r in range(NCORE):
                        nc.sync.dma_start(
                            out=h1T_gath[r * C : (r + 1) * C, :],
                            in_=h1T_shard[:],
                        )
                else:
                    nc.gpsimd.collective_compute(
                        "AllGather",
                        ALU.bypass,
                        replica_groups=rg,
                        ins=[h1T_shard[:].opt()],
                        outs=[h1T_gath[:].opt()],
                    )

                # ================= Phase 3: QKV for own heads, all tokens =======
                # qT/kT columns in natural token order: block (b, j) at
                # col b*T + j*BLK. vv index natural: vi = b*NB + j.
                attn_ctx = tc.tile_pool(name="attn_res", bufs=1)
                attn_res = attn_ctx.__enter__()
                qT_sb = attn_res.tile([128, B * T], BF, tag="qT")
                kT_sb = attn_res.tile([128, B * T], BF, tag="kT")
                vv_sb = attn_res.tile([128, 2 * NB, 130], BF, tag="vv")
                # ones columns for the row-sum trick (cols 64 and 129)
                nc.vector.memset(
                    vv_sb[:].rearrange("p a (h s) -> p a h s", s=65)[:, :, :, 64:65],
                    1.0,
                )
                st_ctx = tc.tile_pool(name="st_ps", bufs=2, space="PSUM")
                st_ps = st_ctx.__enter__()
                pt_ctx = tc.tile_pool(name="pt", bufs=28)
                pt_pool = pt_ctx.__enter__()

                def emit_st_chunk(b, qc):
                    """ST + exp (+ diag mask) for all key blocks of chunk."""
                    pts = []
                    qbase = b * T + qc * 512
                    for j in range(4 * qc + 4):
                        moff = (j - 4 * qc) * 128 if j >= 4 * qc else 0
                        st = st_ps.tile([128, 2, 512], FP, tag="st")
                        for hx in range(2):
                            hs = slice(hx * HD, (hx + 1) * HD)
                            nc.tensor.matmul(
                                st[:, hx, moff:],
                                kT_sb[hs, b * T + j * BLK : b * T + (j + 1) * BLK],
                                qT_sb[hs, qbase + moff : qbase + 512],
                                start=True, stop=True,
                                tile_position=(hx * HD, 0),
                            )
                        # chunk qc=0 tiles live across the batch boundary ->
                        # dedicated ring so the main ring can't deadlock
                        pt = pt_pool.tile(
                            [128, 2, 512], BF,
                            tag="pt0" if qc == 0 else "pt",
                            bufs=8 if qc == 0 else None,
                            name="pt0" if qc == 0 else "pt",
                        )
                        nc.scalar.activation(
                            out=pt[:, :, moff:], in_=st[:, :, moff:],
                            func=AF.Exp, scale=0.125,
                        )
                        if j >= 4 * qc:
                            nc.vector.tensor_mul(
                                out=pt[:, :, moff : moff + BLK],
                                in0=pt[:, :, moff : moff + BLK],
                                in1=utri2_sb,
                            )
                        pts.append(pt)
                    return pts

                pts_c0 = {}
                with (
                    tc.tile_pool(name="wqkv", bufs=1) as wqkv,
                    tc.tile_pool(name="h1Tin", bufs=16) as h1Tin,
                    tc.tile_pool(name="qkv_ps", bufs=1, space="PSUM") as qkv_ps,
                ):
                    wq_sb = wqkv.tile([128, 8, 2 * HD], BF, tag="wq")
                    nc.gpsimd.dma_start(
                        out=wq_sb, in_=wq[:].rearrange("(a p) m -> p a m", p=128)
                    )
                    wk_sb = wqkv.tile([128, 8, 2 * HD], BF, tag="wk")
                    nc.gpsimd.dma_start(
                        out=wk_sb, in_=wk[:].rearrange("(a p) m -> p a m", p=128)
                    )
                    wv_sb = wqkv.tile([128, 8, 2 * HD], BF, tag="wv")
                    nc.gpsimd.dma_start(
                        out=wv_sb, in_=wv[:].rearrange("(a p) m -> p a m", p=128)
                    )

                    for r in range(NCORE):
                        blocks = [(b, j) for b in range(B) for j in (r, NB - 1 - r)]
                        hts = []
                        for ct in range(8):
                            ht = h1Tin.tile([128, OWN], BF, tag="ht", name=f"ht{r}_{ct}")
                            goff = r * C + ct * 128
                            nc.sync.dma_start(
                                out=ht, in_=h1T_gath[goff : goff + 128, :]
                            )
                            hts.append(ht)
                        q_ps = qkv_ps.tile([128, OWN], FP, tag="q_ps")
                        for ct in range(8):
                            nc.tensor.matmul(
                                q_ps, wq_sb[:, ct, :], hts[ct],
                                start=(ct == 0), stop=(ct == 7),
                            )
                        for s, (b, j) in enumerate(blocks):
                            nc.vector.tensor_copy(
                                out=qT_sb[:, b * T + j * BLK : b * T + (j + 1) * BLK],
                                in_=q_ps[:, s * 128 : (s + 1) * 128],
                            )
                        k_ps = qkv_ps.tile([128, OWN], FP, tag="k_ps")
                        for ct in range(8):
                            nc.tensor.matmul(
                                k_ps, wk_sb[:, ct, :], hts[ct],
                                start=(ct == 0), stop=(ct == 7),
                            )
                        for s, (b, j) in enumerate(blocks):
                            nc.vector.tensor_copy(
                                out=kT_sb[:, b * T + j * BLK : b * T + (j + 1) * BLK],
                                in_=k_ps[:, s * 128 : (s + 1) * 128],
                            )
                        for s, (b, j) in enumerate(blocks):
                            v_ps = qkv_ps.tile([128, 2 * HD], FP, tag="v_ps", bufs=2)
                            for ct in range(8):
                                nc.tensor.matmul(
                                    v_ps,
                                    hts[ct][:, s * 128 : (s + 1) * 128],
                                    wv_sb[:, ct, :],
                                    start=(ct == 0), stop=(ct == 7),
                                )
                            vi = b * NB + j
                            nc.vector.tensor_copy(
                                out=vv_sb[:, vi, :]
                                .rearrange("p (h s) -> p h s", s=65)[:, :, 0:64],
                                in_=v_ps[:].rearrange("p (h s) -> p h s", s=64),
                            )
                        if r == 3:
                            # blocks 0-3 of both batches are ready: overlap the
                            # first attention chunks' ST/exp with ranks 4-7
                            pts_c0[0] = emit_st_chunk(0, 0)
                            pts_c0[1] = emit_st_chunk(1, 0)

                # ---- W1/Wproj prefetch: emitted after the gather reads so the
                # sync DMA ring services the latency-critical hts first; the
                # 10 MB lands in SBUF during attention.
                # ut 0-15 resident across both up halves; ut 16-31 stream
                # per-half through an 8-buffer ring (re-read, saves 16KB SBUF)
                w1_sb = []
                for ut in range(16):
                    w1t = w1_pool.tile([128, 8, 128], BF, tag="w1", name=f"w1_{ut}")
                    nc.sync.dma_start(
                        out=w1t,
                        in_=w1b[ut, :, :].rearrange("(a p) m -> p a m", p=128),
                    )
                    w1_sb.append(w1t)
                wp_sb = []
                for ct in range(8):
                    wp = wp_pool.tile([128, C], BF, tag="wp", name=f"wp{ct}")
                    nc.sync.dma_start(
                        out=wp, in_=wproj[ct * 128 : (ct + 1) * 128, :]
                    )
                    wp_sb.append(wp)

                # ============= Phase 4: causal attention, own heads =============
                # Chunked: per (batch, 512-col q-chunk qc), ST for key block j
                # covers q columns [moff, 512) of the chunk (causal trim); both
                # heads row-tiled into a [128, 2, 512] PSUM pair, exp'd in one
                # activation, diagonal blocks masked post-exp.
                with (
                    tc.tile_pool(name="o_ps", bufs=2, space="PSUM") as o_ps_pool,
                    tc.tile_pool(name="tp4", bufs=2, space="PSUM") as tp4_ps,
                    tc.tile_pool(name="oblk", bufs=4) as oblk_pool,
                    tc.tile_pool(name="otsb", bufs=4) as ot_pool,
                ):
                    def emit_pv_chunk(b, qc, pts):
                        """P@V + normalize + transpose + stage for a2a."""
                        for jq in range(4 * qc, 4 * qc + 4):
                            toff = (jq - 4 * qc) * BLK
                            o_ps = o_ps_pool.tile([128, 2, 65], FP, tag="o_ps")
                            for hx in range(2):
                                for j in range(jq + 1):
                                    nc.tensor.matmul(
                                        o_ps[:, hx, :],
                                        pts[j][:, hx, toff : toff + BLK],
                                        vv_sb[:, b * NB + j, hx * 65 : hx * 65 + 65],
                                        start=(j == 0), stop=(j == jq),
                                    )
                            recip2 = stats.tile([128, 2], FP, tag="recip2")
                            nc.vector.reciprocal(out=recip2, in_=o_ps[:, :, 64:65])
                            oblk = oblk_pool.tile([128, 128], FP, tag="oblk")
                            for hx in range(2):
                                nc.vector.tensor_scalar_mul(
                                    out=oblk[:, hx * HD : (hx + 1) * HD],
                                    in0=o_ps[:, hx, 0:HD],
                                    scalar1=recip2[:, hx : hx + 1],
                                )
                            tp = tp4_ps.tile([128, 128], FP, tag="tp", name="tp4")
                            nc.tensor.transpose(tp, oblk, ident_sb)
                            ot = ot_pool.tile([128, 128], BF, tag="ot")
                            nc.vector.tensor_copy(out=ot, in_=tp)
                            rt = _rank_of(jq)
                            co = 0 if jq < NCORE else BLK
                            nc.gpsimd.dma_start(
                                out=a2a_in[b][rt * BLK : (rt + 1) * BLK, co : co + BLK],
                                in_=ot,
                            )

                    og0_sb = []
                    for b in range(B):
                        # software pipeline: ST of chunk k+1 is emitted before
                        # PV of chunk k so PE stays ahead of the ACT exps
                        pts_by_chunk = [pts_c0[b]]
                        for qc in range(4):
                            if qc < 3:
                                pts_by_chunk.append(emit_st_chunk(b, qc + 1))
                            emit_pv_chunk(b, qc, pts_by_chunk[qc])
                        # -------- AllToAll for this batch ----------------------
                        if SIM_MODE:
                            for r in range(NCORE):
                                nc.sync.dma_start(
                                    out=a2a_out[b][r * BLK : (r + 1) * BLK, :],
                                    in_=a2a_in[b][r * BLK : (r + 1) * BLK, :],
                                )
                        else:
                            nc.gpsimd.collective_compute(
                                "AllToAll",
                                ALU.bypass,
                                replica_groups=rg,
                                ins=[a2a_in[b][:].opt()],
                                outs=[a2a_out[b][:].opt()],
                            )

                pt_ctx.__exit__(None, None, None)
                st_ctx.__exit__(None, None, None)
                attn_ctx.__exit__(None, None, None)

                # ================= Phase 5: proj + LN2 + MLP on own tokens ======
                uT_ctx = tc.tile_pool(name="uT", bufs=32)
                uT_pool = uT_ctx.__enter__()
                with (
                    tc.tile_pool(name="mm_ps", bufs=4, space="PSUM") as mm_ps,
                    tc.tile_pool(name="tp5", bufs=2, space="PSUM") as tp5_ps,
                    tc.tile_pool(name="h2Tp", bufs=8) as h2T_pool,
                    tc.tile_pool(name="oTg", bufs=8) as oTg_pool,
                    tc.tile_pool(name="hwork2", bufs=4) as hwork2,
                ):
                    # --- attention outputs for both batches ---
                    for ct in range(8):
                        og = og0_pool.tile(
                            [128, 2 * BLK], BF, tag="og0", name=f"og0_{ct}"
                        )
                        nc.sync.dma_start(
                            out=og, in_=a2a_out[0][ct * 128 : (ct + 1) * 128, :]
                        )
                        og0_sb.append(og)
                    og1_sb = []
                    for ct in range(8):
                        og1 = oTg_pool.tile(
                            [128, 2 * BLK], BF, tag="og1", name=f"og1_{ct}"
                        )
                        nc.sync.dma_start(
                            out=og1, in_=a2a_out[1][ct * 128 : (ct + 1) * 128, :]
                        )
                        og1_sb.append(og1)
                    h2T_sb = [
                        h2T_pool.tile([128, OWN], BF, tag="h2T", name=f"h2T{ct}")
                        for ct in range(8)
                    ]

                    def emit_proj_ln2(tq):
                        ogs = og0_sb if tq < 2 else og1_sb
                        tql = tq % 2
                        for co in range(2):
                            ps = mm_ps.tile([128, 512], FP, tag="mm")
                            for ct in range(8):
                                nc.tensor.matmul(
                                    ps,
                                    ogs[ct][:, tql * 128 : (tql + 1) * 128],
                                    wp_sb[ct][:, co * 512 : (co + 1) * 512],
                                    start=(ct == 0), stop=(ct == 7),
                                )
                            csl = slice(co * 512, (co + 1) * 512)
                            nc.vector.tensor_add(
                                out=xo_sb[tq][:, csl], in0=xo_sb[tq][:, csl], in1=ps
                            )
                            if add_bproj:
                                nc.vector.tensor_add(
                                    out=xo_sb[tq][:, csl],
                                    in0=xo_sb[tq][:, csl],
                                    in1=bprojb[:, csl],
                                )
                        h2 = hwork2.tile([128, C], FP, tag="h2", name=f"h2_{tq}")
                        _layernorm(nc, stats, eps_sb, h2[:], xo_sb[tq][:], g2b, be2b)
                        for ct in range(8):
                            tp = tp5_ps.tile([128, 128], FP, tag="tp", name="tp5")
                            nc.tensor.transpose(
                                tp, h2[:, ct * 128 : (ct + 1) * 128], ident_sb
                            )
                            dst = h2T_sb[ct][:, tq * 128 : (tq + 1) * 128]
                            if ct % 2 == 0:
                                nc.vector.tensor_copy(out=dst, in_=tp)
                            else:
                                nc.scalar.copy(out=dst, in_=tp)

                    uT_sb = []
                    for ut in range(32):
                        u = uT_pool.tile([128, OWN], BF, tag="uT", name=f"uT{ut}")
                        uT_sb.append(u)

                    def emit_up_half(half):
                        hsl = slice(half * 256, (half + 1) * 256)
                        for ut in range(32):
                            if ut < 16:
                                w1t = w1_sb[ut]
                            else:
                                w1t = w1_pool.tile(
                                    [128, 8, 128], BF, tag="w1hi", bufs=8,
                                    name=f"w1hi_{half}_{ut}",
                                )
                                nc.sync.dma_start(
                                    out=w1t,
                                    in_=w1b[ut, :, :].rearrange(
                                        "(a p) m -> p a m", p=128
                                    ),
                                )
                            ups = mm_ps.tile([128, 256], FP, tag="mm")
                            for ct in range(8):
                                nc.tensor.matmul(
                                    ups, w1t[:, ct, :], h2T_sb[ct][:, hsl],
                                    start=(ct == 0), stop=(ct == 7),
                                )
                            nc.scalar.activation(
                                out=uT_sb[ut][:, hsl], in_=ups, func=AF.Relu,
                                bias=b1_sb[:, ut : ut + 1],
                            )

                    # --- proj + LN2 for batch-0 tiles, then MLP-up on that
                    # half (emitted before the batch-1 proj so PE does not
                    # sit in-order behind the second AllToAll), then batch 1.
                    emit_proj_ln2(0)
                    emit_proj_ln2(1)
                    emit_up_half(0)
                    emit_proj_ln2(2)
                    emit_proj_ln2(3)
                    emit_up_half(1)

                # --- MLP down + residual: out = x2 + uT.T @ W2 (+ b2) ---
                # All 32 k-tiles accumulate into 8 persistent PSUM banks.
                with (
                    tc.tile_pool(name="dn_ps", bufs=1, space="PSUM") as dn_ps,
                    tc.tile_pool(name="w2s", bufs=16) as w2_pool,
                ):
                    dn = [
                        dn_ps.tile([128, 512], FP, tag=f"dn{i}", name=f"dn{i}")
                        for i in range(8)
                    ]
                    w2_sb = []
                    for ut in range(32):
                        w2t = w2_pool.tile([128, C], BF, tag="w2", name=f"w2_{ut}")
                        nc.sync.dma_start(
                            out=w2t, in_=w2[ut * 128 : (ut + 1) * 128, :]
                        )
                        w2_sb.append(w2t)
                    for ut in range(32):
                        for tq in range(4):
                            for co in range(2):
                                nc.tensor.matmul(
                                    dn[tq * 2 + co],
                                    uT_sb[ut][:, tq * 128 : (tq + 1) * 128],
                                    w2_sb[ut][:, co * 512 : (co + 1) * 512],
                                    start=(ut == 0), stop=(ut == 31),
                                )
                    for tq in range(4):
                        for co in range(2):
                            csl = slice(co * 512, (co + 1) * 512)
                            nc.vector.tensor_add(
                                out=xo_sb[tq][:, csl],
                                in0=xo_sb[tq][:, csl],
                                in1=dn[tq * 2 + co],
                            )
                        if add_b2:
                            nc.vector.tensor_add(
                                out=xo_sb[tq], in0=xo_sb[tq], in1=b2b
                            )
                        nc.sync.dma_start(
                            out=out[tq * 128 : (tq + 1) * 128, :], in_=xo_sb[tq]
                        )
                uT_ctx.__exit__(None, None, None)
                og0_ctx.__exit__(None, None, None)
                wp_ctx.__exit__(None, None, None)
                w1_ctx.__exit__(None, None, None)

            for _rep in range(reps):
                _body(_rep)

    nc.compile()
    return nc


def _prep_inputs(inputs):
    """Host-side prep: returns per-core in_maps."""
    f32 = lambda a: np.ascontiguousarray(np.asarray(a, dtype=np.float32))
    bf = lambda a: np.ascontiguousarray(np.asarray(a, dtype=np.float32).astype(NPBF))
    x = f32(inputs["x"])
    Wq = np.asarray(inputs["Wq"], np.float32).transpose(1, 0, 2).reshape(C, C)
    Wk = np.asarray(inputs["Wk"], np.float32).transpose(1, 0, 2).reshape(C, C)
    Wv = np.asarray(inputs["Wv"], np.float32).transpose(1, 0, 2).reshape(C, C)
    Wproj = np.asarray(inputs["Wproj"], np.float32)
    W1 = np.asarray(inputs["W1"], np.float32)
    W2 = np.asarray(inputs["W2"], np.float32)

    # permute Wproj rows into gathered-o^T channel order (rank-major heads)
    perm = np.concatenate(
        [np.r_[r * HD : (r + 1) * HD, (r + 8) * HD : (r + 9) * HD] for r in range(8)]
    )
    Wproj_p = bf(Wproj[perm, :])
    W1b = bf(W1.reshape(C, 32, 128).transpose(1, 0, 2))
    b1v = np.ascontiguousarray(np.asarray(inputs["b1"], np.float32).reshape(32, 128))
    utri_m = np.ascontiguousarray(np.triu(np.ones((BLK, BLK), np.float32)).astype(NPBF))
    ident_m = np.ascontiguousarray(np.eye(BLK, dtype=np.float32))

    def pack(sections, arrays, np_dtype):
        buf = np.zeros((sections.total,), dtype=np_dtype)
        for name, a in arrays.items():
            off, shape = sections.offsets[name]
            a = np.asarray(a)
            assert tuple(a.shape) == shape, (name, a.shape, shape)
            buf[off : off + a.size] = a.reshape(-1)
        return buf

    common_f32 = dict(
        b1v=b1v,
        bproj=f32(inputs["bproj"]), b2=f32(inputs["b2"]),
        g1=f32(inputs["g1"]), be1=f32(inputs["be1"]),
        g2=f32(inputs["g2"]), be2=f32(inputs["be2"]),
        ident=ident_m,
    )
    common_bf16 = dict(wproj=Wproj_p, w1b=W1b, w2=bf(W2), utri=utri_m)
    in_maps = []
    for c in range(NCORE):
        hcols = np.r_[c * HD : (c + 1) * HD, (c + 8) * HD : (c + 9) * HD]
        x_own = np.ascontiguousarray(
            np.concatenate([x[b, j * BLK : (j + 1) * BLK, :] for b, j in _own_blocks(c)])
        )
        in_maps.append(
            dict(
                packf32=pack(F32S, dict(common_f32, x_own=x_own), np.float32),
                packbf16=pack(
                    BF16S,
                    dict(
                        common_bf16,
                        wq=bf(Wq[:, hcols]),
                        wk=bf(Wk[:, hcols]),
                        wv=bf(Wv[:, hcols]),
                    ),
                    NPBF,
                ),
            )
        )
    return in_maps


def kernel(**inputs):
    global LAST_RESULT
    in_maps = _prep_inputs(inputs)
    f32v = lambda k: np.asarray(inputs[k], np.float32)
    nc = _build(
        ln1_affine=not (np.all(f32v("g1") == 1) and np.all(f32v("be1") == 0)),
        ln2_affine=not (np.all(f32v("g2") == 1) and np.all(f32v("be2") == 0)),
        add_b2=not np.all(f32v("b2") == 0),
        add_bproj=not np.all(f32v("bproj") == 0),
    )
    res = run_bass_kernel_spmd(
        nc, in_maps, core_ids=list(range(NCORE)), trace=TRACE
    )
    LAST_RESULT = res
    out = np.empty((B, T, C), dtype=np.float32)
    for c in range(NCORE):
        shard = res.results[c]["out"]
        for i, (b, j) in enumerate(_own_blocks(c)):
            out[b, j * BLK : (j + 1) * BLK, :] = shard[i * BLK : (i + 1) * BLK, :]
    return out



# revision 30
# speedup vs baseline: 7.5091x; 7.5091x over previous
"""Trainium2 Bass kernel for a dense transformer block (B=2, T=2048, C=1024, H=16).

Sharding (8 NeuronCores, one chip; identical instruction stream per core,
per-core differences enter only through input data):
  - LayerNorms / projections / MLP: token-sharded. 4096 tokens -> 512 per core.
    Core c owns 128-token blocks {c, 15-c} of each batch (causal load balance).
  - Attention: head-sharded. Core c computes heads {c, c+8} for both batches
    over the full causal sequence.
  - Collective glue: one AllGather of h1^T (post-LN1 activations) and a
    per-batch AllToAll of o^T (attention output) so the second batch's
    attention hides the first AllToAll.

Structure notes (vs the v1 baseline, ~900us -> ~570us best observed):
  - qT/kT/vv are stored in natural token order (batch-major), so attention
    scores stream 512 query columns per matmul: ST[s, t-chunk] = K_j^T Q_chunk
    with the two heads packed as row-tiled (tile_position) K=64 pairs.
  - exp runs on [128, 2, 512-moff] PSUM tiles (two banks, both heads) in one
    activation instruction, trimmed to the causal column range; ~80 big exps
    instead of 544 [128,128] ones. The scalar engine was the v1 bottleneck.
  - P^T tiles live in SBUF per (batch, chunk); P@V consumes only causal
    column slices, so non-causal garbage is never read.
  - MLP-down accumulates all 32 k-tiles in 8 persistent PSUM banks (one per
    output (tq,co) tile): 8 residual adds instead of 32.
  - W1 is prefetched during attention; W2 streams in during the MLP with a
    16-tile window; both are emitted on the sync queue after the
    latency-critical gather reads.
  - The first attention chunk of each batch (ST+exp) is emitted inside the
    QKV rank loop (after rank 3) so the scalar engine starts exping while
    the tensor engine finishes QKV; MLP-up is split by batch token-half and
    emitted before the batch-1 projection so PE is not queued in-order
    behind the second AllToAll.

Precision: matmul operands bf16 (fp32 accumulation in PSUM); LayerNorm
statistics, softmax normalization and both residual streams stay fp32.
"""

import sys

if "/opt/trn_rl_repo" not in sys.path:
    sys.path.insert(0, "/opt/trn_rl_repo")

import ml_dtypes
import numpy as np

import concourse.bass as bass
import concourse.mybir as mybir
import concourse.tile as tile
from concourse import bacc
from concourse.bass_utils import run_bass_kernel_spmd

FP = mybir.dt.float32
BF = mybir.dt.bfloat16
NPBF = ml_dtypes.bfloat16
AF = mybir.ActivationFunctionType
ALU = mybir.AluOpType

B, T, C, H, HD = 2, 2048, 1024, 16, 64
NCORE = 8
BLK = 128
NB = T // BLK  # 16 blocks of 128 tokens per batch
OWN = B * T // NCORE  # 512 tokens per core
EPS = 1e-5

# Optional knobs for the local test harness (not used by grader)
TRACE = False
LAST_RESULT = None
SIM_MODE = False  # replace collectives with local DMA copies (TimelineSim)


def _own_blocks(c):
    """Blocks (b, j) owned by core c, in shard-row order."""
    return [(b, j) for b in range(B) for j in (c, NB - 1 - c)]


def _rank_of(j):
    return j if j < NCORE else NB - 1 - j


def _bcast(handle, n_free):
    """AP broadcasting a 1-D DRAM tensor across 128 partitions (DMA only)."""
    ap = handle[:]
    return bass.AP(tensor=ap.tensor, offset=ap.offset, ap=[[0, 128], *ap.ap])


def _layernorm(nc, pool_stats, eps_sb, out_ap, in_ap, g_sb, be_sb):
    """LN over free axis (1024) of a [128, 1024] tile; out may alias in_."""
    x3 = in_ap.rearrange("p (n s) -> p n s", s=512)
    stats = pool_stats.tile([128, 2, 6], FP, tag="bnstats")
    for sg in range(2):
        nc.vector.bn_stats(out=stats[:, sg, :], in_=x3[:, sg, :])
    mv = pool_stats.tile([128, 2], FP, tag="bnaggr")
    nc.vector.bn_aggr(out=mv, in_=stats)
    std = pool_stats.tile([128, 1], FP, tag="std")
    nc.scalar.activation(out=std, in_=mv[:, 1:2], func=AF.Sqrt, bias=eps_sb)
    rstd = pool_stats.tile([128, 1], FP, tag="rstd")
    nc.vector.reciprocal(out=rstd, in_=std)
    nc.vector.tensor_scalar(
        out=out_ap,
        in0=in_ap,
        scalar1=mv[:, 0:1],
        scalar2=rstd,
        op0=ALU.subtract,
        op1=ALU.mult,
    )
    if g_sb is not None:
        nc.vector.tensor_mul(out=out_ap, in0=out_ap, in1=g_sb)
    if be_sb is not None:
        nc.vector.tensor_add(out=out_ap, in0=out_ap, in1=be_sb)


class _Sections:
    """Byte-compatible packing of many logical tensors into one flat DRAM
    tensor (cuts per-call PJRT buffer-binding overhead, ~40us per input)."""

    def __init__(self):
        self.offsets = {}
        self.total = 0

    def add(self, name, shape):
        n = int(np.prod(shape))
        self.offsets[name] = (self.total, tuple(shape))
        # pad each section to a 512-element boundary for DMA alignment
        self.total += (n + 511) // 512 * 512


F32S = _Sections()
F32S.add("x_own", (OWN, C))
F32S.add("b1v", (32, 128))
F32S.add("bproj", (C,))
F32S.add("b2", (C,))
F32S.add("g1", (C,))
F32S.add("be1", (C,))
F32S.add("g2", (C,))
F32S.add("be2", (C,))

BF16S = _Sections()
BF16S.add("ident", (BLK, BLK))
BF16S.add("wq", (C, 2 * HD))
BF16S.add("wk", (C, 2 * HD))
BF16S.add("wv", (C, 2 * HD))
BF16S.add("wproj", (C, C))
BF16S.add("w1b", (32, C, 128))
BF16S.add("w2", (4 * C, C))
BF16S.add("utri", (BLK, BLK))


def _sect(pack, sections, name):
    """AP over a packed section, reshaped to its logical 2D/3D shape."""
    off, shape = sections.offsets[name]
    n = int(np.prod(shape))
    flat = pack[off : off + n]
    if len(shape) == 1:
        return flat
    if len(shape) == 2:
        return flat.rearrange("(p m) -> p m", p=shape[0])
    return flat.rearrange("(a p m) -> a p m", a=shape[0], p=shape[1])


def _build(reps=1, ln1_affine=True, ln2_affine=True, add_b2=True, add_bproj=True):
    nc = bacc.Bacc(None, num_devices=NCORE)

    # ---- kernel I/O (per-core data differs, shapes identical) ----
    packf32 = nc.dram_tensor("packf32", [F32S.total], FP, kind="ExternalInput")
    packbf16 = nc.dram_tensor("packbf16", [BF16S.total], BF, kind="ExternalInput")
    x_own = _sect(packf32, F32S, "x_own")
    b1v = _sect(packf32, F32S, "b1v")
    bproj = _sect(packf32, F32S, "bproj")
    b2 = _sect(packf32, F32S, "b2")
    g1 = _sect(packf32, F32S, "g1")
    be1 = _sect(packf32, F32S, "be1")
    g2 = _sect(packf32, F32S, "g2")
    be2 = _sect(packf32, F32S, "be2")
    ident = _sect(packbf16, BF16S, "ident")
    wq = _sect(packbf16, BF16S, "wq")
    wk = _sect(packbf16, BF16S, "wk")
    wv = _sect(packbf16, BF16S, "wv")
    wproj = _sect(packbf16, BF16S, "wproj")
    w1b = _sect(packbf16, BF16S, "w1b")
    w2 = _sect(packbf16, BF16S, "w2")
    utri = _sect(packbf16, BF16S, "utri")
    out = nc.dram_tensor("out", [OWN, C], FP, kind="ExternalOutput")

    rg = [list(range(NCORE))]

    with tile.TileContext(nc) as tc:
        with (
            tc.tile_pool(name="dram", bufs=1, space="DRAM") as dram,
            tc.tile_pool(name="consts", bufs=1) as consts,
            tc.tile_pool(name="stats", bufs=12) as stats,
            tc.tile_pool(name="resid", bufs=4) as resid,
        ):

            # ---- constants in SBUF ----
            # Only ident is loaded up front (needed by the first transposes);
            # everything else is deferred so the x_own reads own the DMA
            # engines at startup.
            ident_sb = consts.tile([BLK, BLK], BF)
            nc.sync.dma_start(out=ident_sb, in_=ident[:])
            eps_sb = consts.tile([128, 1], FP)
            nc.vector.memset(eps_sb, EPS)
            g1b = be1b = g2b = be2b = None
            if ln1_affine:
                g1b = consts.tile([128, C], FP, name="g1b")
                nc.gpsimd.dma_start(out=g1b, in_=_bcast(g1, C))
                be1b = consts.tile([128, C], FP, name="be1b")
                nc.gpsimd.dma_start(out=be1b, in_=_bcast(be1, C))
            if ln2_affine:
                g2b = consts.tile([128, C], FP, name="g2b")
                nc.gpsimd.dma_start(out=g2b, in_=_bcast(g2, C))
                be2b = consts.tile([128, C], FP, name="be2b")
                nc.gpsimd.dma_start(out=be2b, in_=_bcast(be2, C))
            bprojb = b2b = None
            if add_bproj:
                bprojb = consts.tile([128, C], FP, name="bprojb")
                nc.gpsimd.dma_start(out=bprojb, in_=_bcast(bproj, C))
            if add_b2:
                b2b = consts.tile([128, C], FP, name="b2b")
                nc.gpsimd.dma_start(out=b2b, in_=_bcast(b2, C))
            utri_sb = consts.tile([BLK, BLK], BF)
            utri2_sb = consts.tile([BLK, 2 * BLK], BF)
            b1_sb = consts.tile([128, 32], FP)

            def _body(rep):
                HC = C // 2
                # Pools whose tiles outlive the attention section are
                # allocated first (LIFO pool release discipline); their DMAs
                # are emitted later, after the latency-critical gather reads.
                w1_ctx = tc.tile_pool(name="w1s", bufs=16)
                w1_pool = w1_ctx.__enter__()
                wp_ctx = tc.tile_pool(name="wp", bufs=8)
                wp_pool = wp_ctx.__enter__()
                og0_ctx = tc.tile_pool(name="og0", bufs=8)
                og0_pool = og0_ctx.__enter__()
                h1T_shard = dram.tile([C, OWN], BF, name=f"h1T_shard{rep}", tag=f"sh{rep}")
                h1T_gath = dram.tile(
                    [NCORE * C, OWN], BF, name=f"h1T_gath{rep}", tag=f"ga{rep}",
                    addr_space="Local" if SIM_MODE else "Shared",
                )
                # Per-batch AllToAll: a2a_in[b] rows r*128.. = my heads' o^T for
                # rank r's tokens of batch b; a2a_out[b] rows r*128.. = rank r's
                # heads for MY tokens of batch b.
                a2a_in = [
                    dram.tile(
                        [NCORE * BLK, 2 * BLK], BF,
                        name=f"a2a_in{rep}_{b}", tag=f"ai{rep}_{b}",
                    )
                    for b in range(B)
                ]
                a2a_out = [
                    dram.tile(
                        [NCORE * BLK, 2 * BLK], BF,
                        name=f"a2a_out{rep}_{b}", tag=f"ao{rep}_{b}",
                    )
                    for b in range(B)
                ]

                # ================= Phase 1: LN1 on own tokens, h1^T shard =======
                xo_sb = []  # own x tiles; overwritten with x2 (post-attn residual)
                for i in range(4):
                    xo = resid.tile([128, C], FP, tag="xo", name=f"xo{i}")
                    xo_sb.append(xo)
                with (
                    tc.tile_pool(name="hwork", bufs=4) as hwork,
                    tc.tile_pool(name="h1Tp", bufs=8) as h1Tp,
                    tc.tile_pool(name="tp1", bufs=2, space="PSUM") as tp1_ps,
                ):
                    h1T_sb = [
                        h1Tp.tile([128, OWN], BF, tag="h1T", name=f"h1T{ct}")
                        for ct in range(8)
                    ]
                    for i in range(4):
                        h1 = hwork.tile([128, C], BF, tag="h1", name=f"h1_{i}")
                        nc.sync.dma_start(
                            out=xo_sb[i], in_=x_own[i * 128 : (i + 1) * 128, :]
                        )
                        _layernorm(nc, stats, eps_sb, h1[:], xo_sb[i][:], g1b, be1b)
                        for ct in range(8):
                            tp = tp1_ps.tile([128, 128], BF, tag="tp", name="tp1")
                            nc.tensor.transpose(
                                tp, h1[:, ct * 128 : (ct + 1) * 128], ident_sb
                            )
                            dst = h1T_sb[ct][:, i * 128 : (i + 1) * 128]
                            if ct % 2 == 0:
                                nc.vector.tensor_copy(out=dst, in_=tp)
                            else:
                                nc.scalar.copy(out=dst, in_=tp)
                    for ct in range(8):
                        nc.sync.dma_start(
                            out=h1T_shard[ct * 128 : (ct + 1) * 128, :], in_=h1T_sb[ct]
                        )
                # causal-mask constants: emitted after the phase-1 reads so
                # the x_own DMAs own the engines at startup
                nc.sync.dma_start(out=utri_sb, in_=utri[:])
                nc.vector.tensor_copy(out=utri2_sb[:, 0:BLK], in_=utri_sb)
                nc.vector.tensor_copy(out=utri2_sb[:, BLK : 2 * BLK], in_=utri_sb)

                # ================= Phase 2: AllGather h1^T ======================
                if SIM_MODE:
                    for r in range(NCORE):
                        nc.sync.dma_start(
                            out=h1T_gath[r * C : (r + 1) * C, :],
                            in_=h1T_shard[:],
                        )
                else:
                    nc.gpsimd.collective_compute(
                        "AllGather",
                        ALU.bypass,
                        replica_groups=rg,
                        ins=[h1T_shard[:].opt()],
                        outs=[h1T_gath[:].opt()],
                    )

                # ================= Phase 3: QKV for own heads, all tokens =======
                # qT/kT columns in natural token order: block (b, j) at
                # col b*T + j*BLK. vv index natural: vi = b*NB + j.
                attn_ctx = tc.tile_pool(name="attn_res", bufs=1)
                attn_res = attn_ctx.__enter__()
                qT_sb = attn_res.tile([128, B * T], BF, tag="qT")
                kT_sb = attn_res.tile([128, B * T], BF, tag="kT")
                vv_sb = attn_res.tile([128, 2 * NB, 130], BF, tag="vv")
                # ones columns for the row-sum trick (cols 64 and 129)
                nc.vector.memset(
                    vv_sb[:].rearrange("p a (h s) -> p a h s", s=65)[:, :, :, 64:65],
                    1.0,
                )
                st_ctx = tc.tile_pool(name="st_ps", bufs=2, space="PSUM")
                st_ps = st_ctx.__enter__()
                pt_ctx = tc.tile_pool(name="pt", bufs=28)
                pt_pool = pt_ctx.__enter__()

                def emit_st_chunk(b, qc):
                    """ST + exp (+ diag mask) for all key blocks of chunk."""
                    pts = []
                    qbase = b * T + qc * 512
                    for j in range(4 * qc + 4):
                        moff = (j - 4 * qc) * 128 if j >= 4 * qc else 0
                        st = st_ps.tile([128, 2, 512], FP, tag="st")
                        for hx in range(2):
                            hs = slice(hx * HD, (hx + 1) * HD)
                            nc.tensor.matmul(
                                st[:, hx, moff:],
                                kT_sb[hs, b * T + j * BLK : b * T + (j + 1) * BLK],
                                qT_sb[hs, qbase + moff : qbase + 512],
                                start=True, stop=True,
                                tile_position=(hx * HD, 0),
                            )
                        # chunk qc=0 tiles live across the batch boundary ->
                        # dedicated ring so the main ring can't deadlock
                        pt = pt_pool.tile(
                            [128, 2, 512], BF,
                            tag="pt0" if qc == 0 else "pt",
                            bufs=8 if qc == 0 else None,
                            name="pt0" if qc == 0 else "pt",
                        )
                        nc.scalar.activation(
                            out=pt[:, :, moff:], in_=st[:, :, moff:],
                            func=AF.Exp, scale=0.125,
                        )
                        if j >= 4 * qc:
                            nc.vector.tensor_mul(
                                out=pt[:, :, moff : moff + BLK],
                                in0=pt[:, :, moff : moff + BLK],
                                in1=utri2_sb,
                            )
                        pts.append(pt)
                    return pts

                pts_c0 = {}
                with (
                    tc.tile_pool(name="wqkv", bufs=1) as wqkv,
                    tc.tile_pool(name="h1Tin", bufs=16) as h1Tin,
                    tc.tile_pool(name="qkv_ps", bufs=1, space="PSUM") as qkv_ps,
                ):
                    # QKV weights on the sync queue, after the shard stores:
                    # they stream during the AllGather and are ready at rank 0
                    wq_sb = wqkv.tile([128, 8, 2 * HD], BF, tag="wq")
                    nc.sync.dma_start(
                        out=wq_sb, in_=wq[:].rearrange("(a p) m -> p a m", p=128)
                    )
                    wk_sb = wqkv.tile([128, 8, 2 * HD], BF, tag="wk")
                    nc.sync.dma_start(
                        out=wk_sb, in_=wk[:].rearrange("(a p) m -> p a m", p=128)
                    )
                    wv_sb = wqkv.tile([128, 8, 2 * HD], BF, tag="wv")
                    nc.sync.dma_start(
                        out=wv_sb, in_=wv[:].rearrange("(a p) m -> p a m", p=128)
                    )

                    for r in range(NCORE):
                        blocks = [(b, j) for b in range(B) for j in (r, NB - 1 - r)]
                        hts = []
                        for ct in range(8):
                            ht = h1Tin.tile([128, OWN], BF, tag="ht", name=f"ht{r}_{ct}")
                            goff = r * C + ct * 128
                            nc.sync.dma_start(
                                out=ht, in_=h1T_gath[goff : goff + 128, :]
                            )
                            hts.append(ht)
                        q_ps = qkv_ps.tile([128, OWN], FP, tag="q_ps")
                        for ct in range(8):
                            nc.tensor.matmul(
                                q_ps, wq_sb[:, ct, :], hts[ct],
                                start=(ct == 0), stop=(ct == 7),
                            )
                        for s, (b, j) in enumerate(blocks):
                            nc.vector.tensor_copy(
                                out=qT_sb[:, b * T + j * BLK : b * T + (j + 1) * BLK],
                                in_=q_ps[:, s * 128 : (s + 1) * 128],
                            )
                        k_ps = qkv_ps.tile([128, OWN], FP, tag="k_ps")
                        for ct in range(8):
                            nc.tensor.matmul(
                                k_ps, wk_sb[:, ct, :], hts[ct],
                                start=(ct == 0), stop=(ct == 7),
                            )
                        for s, (b, j) in enumerate(blocks):
                            nc.vector.tensor_copy(
                                out=kT_sb[:, b * T + j * BLK : b * T + (j + 1) * BLK],
                                in_=k_ps[:, s * 128 : (s + 1) * 128],
                            )
                        for s, (b, j) in enumerate(blocks):
                            v_ps = qkv_ps.tile([128, 2 * HD], FP, tag="v_ps", bufs=2)
                            for ct in range(8):
                                nc.tensor.matmul(
                                    v_ps,
                                    hts[ct][:, s * 128 : (s + 1) * 128],
                                    wv_sb[:, ct, :],
                                    start=(ct == 0), stop=(ct == 7),
                                )
                            vi = b * NB + j
                            nc.vector.tensor_copy(
                                out=vv_sb[:, vi, :]
                                .rearrange("p (h s) -> p h s", s=65)[:, :, 0:64],
                                in_=v_ps[:].rearrange("p (h s) -> p h s", s=64),
                            )
                        if r == 3:
                            # blocks 0-3 of both batches are ready: overlap the
                            # first attention chunks' ST/exp with ranks 4-7
                            pts_c0[0] = emit_st_chunk(0, 0)
                            pts_c0[1] = emit_st_chunk(1, 0)

                # ---- W1/Wproj/b1 prefetch: emitted after the gather reads so
                # the sync DMA ring services the latency-critical hts first;
                # the 10 MB lands in SBUF during QKV/attention.
                # ut 0-15 resident across both up halves; ut 16-31 stream
                # per-half through an 8-buffer ring (re-read, saves 16KB SBUF)
                w1_sb = []
                for ut in range(16):
                    w1t = w1_pool.tile([128, 8, 128], BF, tag="w1", name=f"w1_{ut}")
                    nc.sync.dma_start(
                        out=w1t,
                        in_=w1b[ut, :, :].rearrange("(a p) m -> p a m", p=128),
                    )
                    w1_sb.append(w1t)
                wp_sb = []
                for ct in range(8):
                    wp = wp_pool.tile([128, C], BF, tag="wp", name=f"wp{ct}")
                    nc.sync.dma_start(
                        out=wp, in_=wproj[ct * 128 : (ct + 1) * 128, :]
                    )
                    wp_sb.append(wp)
                nc.sync.dma_start(out=b1_sb, in_=b1v[:].rearrange("a p -> p a"))

                # ============= Phase 4: causal attention, own heads =============
                # Chunked: per (batch, 512-col q-chunk qc), ST for key block j
                # covers q columns [moff, 512) of the chunk (causal trim); both
                # heads row-tiled into a [128, 2, 512] PSUM pair, exp'd in one
                # activation, diagonal blocks masked post-exp.
                # Right-side pools persist into phase 5 (independent LIFO
                # stack) so batch-0 proj/LN2 can interleave into batch-1's
                # ACT-bound attention stream.
                mm_ctx = tc.tile_pool(name="mm_ps", bufs=2, space="PSUM", side="right")
                mm_ps = mm_ctx.__enter__()
                tp5_ctx = tc.tile_pool(name="tp5", bufs=2, space="PSUM", side="right")
                tp5_ps = tp5_ctx.__enter__()
                h2T_ctx = tc.tile_pool(name="h2Tp", bufs=8, side="right")
                h2T_pool = h2T_ctx.__enter__()
                hwork2_ctx = tc.tile_pool(name="hwork2", bufs=2, side="right")
                hwork2 = hwork2_ctx.__enter__()
                og0_sb = []
                og1_sb = []
                h2T_sb = [
                    h2T_pool.tile([128, OWN], BF, tag="h2T", name=f"h2T{ct}")
                    for ct in range(8)
                ]

                def emit_proj_ln2(tq):
                    ogs = og0_sb if tq < 2 else og1_sb
                    tql = tq % 2
                    for co in range(2):
                        ps = mm_ps.tile([128, 512], FP, tag="mm")
                        for ct in range(8):
                            nc.tensor.matmul(
                                ps,
                                ogs[ct][:, tql * 128 : (tql + 1) * 128],
                                wp_sb[ct][:, co * 512 : (co + 1) * 512],
                                start=(ct == 0), stop=(ct == 7),
                            )
                        csl = slice(co * 512, (co + 1) * 512)
                        nc.vector.tensor_add(
                            out=xo_sb[tq][:, csl], in0=xo_sb[tq][:, csl], in1=ps
                        )
                        if add_bproj:
                            nc.vector.tensor_add(
                                out=xo_sb[tq][:, csl],
                                in0=xo_sb[tq][:, csl],
                                in1=bprojb[:, csl],
                            )
                    h2 = hwork2.tile([128, C], BF, tag="h2", name=f"h2_{tq}")
                    _layernorm(nc, stats, eps_sb, h2[:], xo_sb[tq][:], g2b, be2b)
                    for ct in range(8):
                        tp = tp5_ps.tile([128, 128], BF, tag="tp", name="tp5")
                        nc.tensor.transpose(
                            tp, h2[:, ct * 128 : (ct + 1) * 128], ident_sb
                        )
                        dst = h2T_sb[ct][:, tq * 128 : (tq + 1) * 128]
                        if ct % 2 == 0:
                            nc.vector.tensor_copy(out=dst, in_=tp)
                        else:
                            nc.scalar.copy(out=dst, in_=tp)

                with (
                    tc.tile_pool(name="o_ps", bufs=2, space="PSUM") as o_ps_pool,
                    tc.tile_pool(name="tp4", bufs=2, space="PSUM") as tp4_ps,
                    tc.tile_pool(name="oblk", bufs=4) as oblk_pool,
                    tc.tile_pool(name="otsb", bufs=4) as ot_pool,
                ):
                    def emit_pv_chunk(b, qc, pts):
                        """P@V + normalize + transpose + stage for a2a."""
                        for jq in range(4 * qc, 4 * qc + 4):
                            toff = (jq - 4 * qc) * BLK
                            o_ps = o_ps_pool.tile([128, 2, 65], FP, tag="o_ps")
                            for hx in range(2):
                                for j in range(jq + 1):
                                    nc.tensor.matmul(
                                        o_ps[:, hx, :],
                                        pts[j][:, hx, toff : toff + BLK],
                                        vv_sb[:, b * NB + j, hx * 65 : hx * 65 + 65],
                                        start=(j == 0), stop=(j == jq),
                                    )
                            recip2 = stats.tile([128, 2], FP, tag="recip2")
                            nc.vector.reciprocal(out=recip2, in_=o_ps[:, :, 64:65])
                            oblk = oblk_pool.tile([128, 128], BF, tag="oblk")
                            for hx in range(2):
                                nc.vector.tensor_scalar_mul(
                                    out=oblk[:, hx * HD : (hx + 1) * HD],
                                    in0=o_ps[:, hx, 0:HD],
                                    scalar1=recip2[:, hx : hx + 1],
                                )
                            tp = tp4_ps.tile([128, 128], BF, tag="tp", name="tp4")
                            nc.tensor.transpose(tp, oblk, ident_sb)
                            ot = ot_pool.tile([128, 128], BF, tag="ot")
                            nc.vector.tensor_copy(out=ot, in_=tp)
                            rt = _rank_of(jq)
                            co = 0 if jq < NCORE else BLK
                            nc.gpsimd.dma_start(
                                out=a2a_in[b][rt * BLK : (rt + 1) * BLK, co : co + BLK],
                                in_=ot,
                            )

                    def emit_a2a(b):
                        if SIM_MODE:
                            for r in range(NCORE):
                                nc.sync.dma_start(
                                    out=a2a_out[b][r * BLK : (r + 1) * BLK, :],
                                    in_=a2a_in[b][r * BLK : (r + 1) * BLK, :],
                                )
                        else:
                            nc.gpsimd.collective_compute(
                                "AllToAll",
                                ALU.bypass,
                                replica_groups=rg,
                                ins=[a2a_in[b][:].opt()],
                                outs=[a2a_out[b][:].opt()],
                            )

                    def attn_batch(b, fillers=()):
                        # software pipeline: ST of chunk k+1 is emitted before
                        # PV of chunk k so PE stays ahead of the ACT exps;
                        # fillers are PE-heavy stages slotted between chunks to
                        # cover the exp-bound stretches.
                        fillers = list(fillers)
                        pts_by_chunk = [pts_c0[b]]
                        for qc in range(4):
                            if qc < 3:
                                pts_by_chunk.append(emit_st_chunk(b, qc + 1))
                            if fillers:
                                fillers.pop(0)()
                            emit_pv_chunk(b, qc, pts_by_chunk[qc])
                        for f in fillers:
                            f()
                        emit_a2a(b)

                    attn_batch(0)
                    # batch-0 attention outputs: reads wait on the AllToAll,
                    # issued now so proj(b0) can interleave into batch 1
                    for ct in range(8):
                        og = og0_pool.tile(
                            [128, 2 * BLK], BF, tag="og0", name=f"og0_{ct}"
                        )
                        nc.sync.dma_start(
                            out=og, in_=a2a_out[0][ct * 128 : (ct + 1) * 128, :]
                        )
                        og0_sb.append(og)
                    attn_batch(
                        1,
                        fillers=[
                            lambda: None,
                            lambda: emit_proj_ln2(0),
                            lambda: emit_proj_ln2(1),
                        ],
                    )

                pt_ctx.__exit__(None, None, None)
                st_ctx.__exit__(None, None, None)
                attn_ctx.__exit__(None, None, None)

                # ================= Phase 5: proj + LN2 + MLP on own tokens ======
                uT_ctx = tc.tile_pool(name="uT", bufs=32)
                uT_pool = uT_ctx.__enter__()
                w2_ctx = tc.tile_pool(name="w2s", bufs=16)
                w2_pool = w2_ctx.__enter__()
                w2_sb = []
                with (
                    tc.tile_pool(name="mm_ps", bufs=4, space="PSUM") as mm_ps,
                    tc.tile_pool(name="tp5", bufs=2, space="PSUM") as tp5_ps,
                    tc.tile_pool(name="h2Tp", bufs=8) as h2T_pool,
                    tc.tile_pool(name="oTg", bufs=8) as oTg_pool,
                    tc.tile_pool(name="hwork2", bufs=4) as hwork2,
                ):
                    # --- attention outputs for both batches ---
                    for ct in range(8):
                        og = og0_pool.tile(
                            [128, 2 * BLK], BF, tag="og0", name=f"og0_{ct}"
                        )
                        nc.sync.dma_start(
                            out=og, in_=a2a_out[0][ct * 128 : (ct + 1) * 128, :]
                        )
                        og0_sb.append(og)
                    og1_sb = []
                    for ct in range(8):
                        og1 = oTg_pool.tile(
                            [128, 2 * BLK], BF, tag="og1", name=f"og1_{ct}"
                        )
                        nc.sync.dma_start(
                            out=og1, in_=a2a_out[1][ct * 128 : (ct + 1) * 128, :]
                        )
                        og1_sb.append(og1)
                    h2T_sb = [
                        h2T_pool.tile([128, OWN], BF, tag="h2T", name=f"h2T{ct}")
                        for ct in range(8)
                    ]

                    def emit_proj_ln2(tq):
                        ogs = og0_sb if tq < 2 else og1_sb
                        tql = tq % 2
                        for co in range(2):
                            ps = mm_ps.tile([128, 512], FP, tag="mm")
                            for ct in range(8):
                                nc.tensor.matmul(
                                    ps,
                                    ogs[ct][:, tql * 128 : (tql + 1) * 128],
                                    wp_sb[ct][:, co * 512 : (co + 1) * 512],
                                    start=(ct == 0), stop=(ct == 7),
                                )
                            csl = slice(co * 512, (co + 1) * 512)
                            nc.vector.tensor_add(
                                out=xo_sb[tq][:, csl], in0=xo_sb[tq][:, csl], in1=ps
                            )
                            if add_bproj:
                                nc.vector.tensor_add(
                                    out=xo_sb[tq][:, csl],
                                    in0=xo_sb[tq][:, csl],
                                    in1=bprojb[:, csl],
                                )
                        h2 = hwork2.tile([128, C], BF, tag="h2", name=f"h2_{tq}")
                        _layernorm(nc, stats, eps_sb, h2[:], xo_sb[tq][:], g2b, be2b)
                        for ct in range(8):
                            tp = tp5_ps.tile([128, 128], BF, tag="tp", name="tp5")
                            nc.tensor.transpose(
                                tp, h2[:, ct * 128 : (ct + 1) * 128], ident_sb
                            )
                            dst = h2T_sb[ct][:, tq * 128 : (tq + 1) * 128]
                            if ct % 2 == 0:
                                nc.vector.tensor_copy(out=dst, in_=tp)
                            else:
                                nc.scalar.copy(out=dst, in_=tp)

                    uT_sb = []
                    for ut in range(32):
                        u = uT_pool.tile([128, OWN], BF, tag="uT", name=f"uT{ut}")
                        uT_sb.append(u)

                    def emit_w2_stream(uts):
                        # w2 streamed on the sync queue (idle during MLP) so
                        # the down matmuls never wait on weights
                        for ut in uts:
                            w2t = w2_pool.tile([128, C], BF, tag="w2", name=f"w2_{ut}")
                            nc.sync.dma_start(
                                out=w2t, in_=w2[ut * 128 : (ut + 1) * 128, :]
                            )
                            w2_sb.append(w2t)

                    def emit_up_half(half):
                        hsl = slice(half * 256, (half + 1) * 256)
                        for ut in range(32):
                            if ut < 16:
                                w1t = w1_sb[ut]
                            else:
                                w1t = w1_pool.tile(
                                    [128, 8, 128], BF, tag="w1hi", bufs=8,
                                    name=f"w1hi_{half}_{ut}",
                                )
                                nc.sync.dma_start(
                                    out=w1t,
                                    in_=w1b[ut, :, :].rearrange(
                                        "(a p) m -> p a m", p=128
                                    ),
                                )
                            ups = mm_ps.tile([128, 256], FP, tag="mm")
                            for ct in range(8):
                                nc.tensor.matmul(
                                    ups, w1t[:, ct, :], h2T_sb[ct][:, hsl],
                                    start=(ct == 0), stop=(ct == 7),
                                )
                            # relu+bias on DVE (frees the ACT engine for exp)
                            nc.vector.tensor_scalar(
                                out=uT_sb[ut][:, hsl],
                                in0=ups,
                                scalar1=b1_sb[:, ut : ut + 1],
                                scalar2=0.0,
                                op0=ALU.add,
                                op1=ALU.max,
                            )

                    # --- proj + LN2 for batch-0 tiles, then MLP-up on that
                    # half (emitted before the batch-1 proj so PE does not
                    # sit in-order behind the second AllToAll), then batch 1.
                    emit_proj_ln2(0)
                    emit_proj_ln2(1)
                    emit_w2_stream(range(8))
                    emit_up_half(0)
                    emit_w2_stream(range(8, 16))
                    emit_proj_ln2(2)
                    emit_proj_ln2(3)
                    emit_up_half(1)
                    emit_w2_stream(range(16, 32))

                # --- MLP down + residual: out = x2 + uT.T @ W2 (+ b2) ---
                # tq-major halves: banks for tq 0-1 finish while tq 2-3 still
                # accumulate, so their residual adds + out DMA overlap the
                # remaining matmuls (shorter serial tail).
                with tc.tile_pool(name="dn_ps", bufs=1, space="PSUM") as dn_ps:
                    dn = [
                        dn_ps.tile([128, 512], FP, tag=f"dn{i}", name=f"dn{i}")
                        for i in range(8)
                    ]

                    def emit_down_tqs(tqs):
                        for ut in range(32):
                            for tq in tqs:
                                for co in range(2):
                                    nc.tensor.matmul(
                                        dn[tq * 2 + co],
                                        uT_sb[ut][:, tq * 128 : (tq + 1) * 128],
                                        w2_sb[ut][:, co * 512 : (co + 1) * 512],
                                        start=(ut == 0), stop=(ut == 31),
                                    )
                        for tq in tqs:
                            for co in range(2):
                                csl = slice(co * 512, (co + 1) * 512)
                                eng = nc.vector
                                eng.tensor_add(
                                    out=xo_sb[tq][:, csl],
                                    in0=xo_sb[tq][:, csl],
                                    in1=dn[tq * 2 + co],
                                )
                            if add_b2:
                                nc.vector.tensor_add(
                                    out=xo_sb[tq], in0=xo_sb[tq], in1=b2b
                                )
                            nc.sync.dma_start(
                                out=out[tq * 128 : (tq + 1) * 128, :], in_=xo_sb[tq]
                            )

                    emit_down_tqs((0, 1))
                    emit_down_tqs((2, 3))
                w2_ctx.__exit__(None, None, None)
                uT_ctx.__exit__(None, None, None)
                og0_ctx.__exit__(None, None, None)
                wp_ctx.__exit__(None, None, None)
                w1_ctx.__exit__(None, None, None)

            for _rep in range(reps):
                _body(_rep)

    nc.compile()
    return nc


def _prep_inputs(inputs):
    """Host-side prep: returns per-core in_maps."""
    f32 = lambda a: np.ascontiguousarray(np.asarray(a, dtype=np.float32))
    bf = lambda a: np.ascontiguousarray(np.asarray(a, dtype=np.float32).astype(NPBF))
    x = f32(inputs["x"])
    Wq = np.asarray(inputs["Wq"], np.float32).transpose(1, 0, 2).reshape(C, C)
    Wk = np.asarray(inputs["Wk"], np.float32).transpose(1, 0, 2).reshape(C, C)
    Wv = np.asarray(inputs["Wv"], np.float32).transpose(1, 0, 2).reshape(C, C)
    Wproj = np.asarray(inputs["Wproj"], np.float32)
    W1 = np.asarray(inputs["W1"], np.float32)
    W2 = np.asarray(inputs["W2"], np.float32)

    # permute Wproj rows into gathered-o^T channel order (rank-major heads)
    perm = np.concatenate(
        [np.r_[r * HD : (r + 1) * HD, (r + 8) * HD : (r + 9) * HD] for r in range(8)]
    )
    Wproj_p = bf(Wproj[perm, :])
    W1b = bf(W1.reshape(C, 32, 128).transpose(1, 0, 2))
    b1v = np.ascontiguousarray(np.asarray(inputs["b1"], np.float32).reshape(32, 128))
    utri_m = np.ascontiguousarray(np.triu(np.ones((BLK, BLK), np.float32)).astype(NPBF))
    ident_m = np.ascontiguousarray(np.eye(BLK, dtype=np.float32).astype(NPBF))

    def pack(sections, arrays, np_dtype):
        buf = np.zeros((sections.total,), dtype=np_dtype)
        for name, a in arrays.items():
            off, shape = sections.offsets[name]
            a = np.asarray(a)
            assert tuple(a.shape) == shape, (name, a.shape, shape)
            buf[off : off + a.size] = a.reshape(-1)
        return buf

    common_f32 = dict(
        b1v=b1v,
        bproj=f32(inputs["bproj"]), b2=f32(inputs["b2"]),
        g1=f32(inputs["g1"]), be1=f32(inputs["be1"]),
        g2=f32(inputs["g2"]), be2=f32(inputs["be2"]),
    )
    common_bf16 = dict(
        wproj=Wproj_p, w1b=W1b, w2=bf(W2), utri=utri_m, ident=ident_m
    )
    in_maps = []
    for c in range(NCORE):
        hcols = np.r_[c * HD : (c + 1) * HD, (c + 8) * HD : (c + 9) * HD]
        x_own = np.ascontiguousarray(
            np.concatenate([x[b, j * BLK : (j + 1) * BLK, :] for b, j in _own_blocks(c)])
        )
        in_maps.append(
            dict(
                packf32=pack(F32S, dict(common_f32, x_own=x_own), np.float32),
                packbf16=pack(
                    BF16S,
                    dict(
                        common_bf16,
                        wq=bf(Wq[:, hcols]),
                        wk=bf(Wk[:, hcols]),
                        wv=bf(Wv[:, hcols]),
                    ),
                    NPBF,
                ),
            )
        )
    return in_maps


def kernel(**inputs):
    global LAST_RESULT
    in_maps = _prep_inputs(inputs)
    f32v = lambda k: np.asarray(inputs[k], np.float32)
    nc = _build(
        ln1_affine=not (np.all(f32v("g1") == 1) and np.all(f32v("be1") == 0)),
        ln2_affine=not (np.all(f32v("g2") == 1) and np.all(f32v("be2") == 0)),
        add_b2=not np.all(f32v("b2") == 0),
        add_bproj=not np.all(f32v("bproj") == 0),
    )
    res = run_bass_kernel_spmd(
        nc, in_maps, core_ids=list(range(NCORE)), trace=TRACE
    )
    LAST_RESULT = res
    out = np.empty((B, T, C), dtype=np.float32)
    for c in range(NCORE):
        shard = res.results[c]["out"]
        for i, (b, j) in enumerate(_own_blocks(c)):
            out[b, j * BLK : (j + 1) * BLK, :] = shard[i * BLK : (i + 1) * BLK, :]
    return out

